# revision 1
# baseline (speedup 1.0000x reference)
"""Trainium2 Bass kernel for the quantized BasicBlock (conv3x3/s2 + fakequant + conv3x3/s1 + fakequant).

Sharding: data-parallel over batch across 8 cores (8 images each), weights replicated.

Device math (per core, B=8):
  conv1: implicit GEMM, 9 taps x 2 ci-blocks, input as bf16 hi+lo split (x = hi + lo
         reconstructs ~16 mantissa bits), integer-valued bf16 weights, fp32 PSUM accum.
  act1:  v = P1*(s_w1/s_a1) + bq1/s_a1; y = clip(rne(v), -128, 127) via the fp32
         magic-number trick on the DVE; y stored as integer-valued bf16 into a
         zero-padded [16x16] layout for conv2.
  conv2: exact integer bf16 GEMM, 9 taps x 4 ci-blocks.
  act2:  v2 = P2*(s_a1*s_w2/s_a2) + bq2/s_a2; out = clip(rne(v2), -128, 127) * s_a2.

Stride-2 conv1 is handled by a host-side phase split: x is scattered into 2x2 parity
planes, zero-padded to 15x15, so each tap reads a stride-1 14x14 window of one plane.

DMAs are split into per-(plane, ci-block) / per-(tap, ci-block) chunks with their own
tiles (dependency granularity) and alternate between the two HWDGE trigger engines
(Sync, Scalar) in first-use order, so the matmul stream starts as soon as tap 0's
weights + plane arrive instead of after the whole input load.
"""
import os
import sys
from contextlib import ExitStack

import numpy as np
import ml_dtypes

for _p in ("/opt/trn_rl_repo",):
    if _p not in sys.path and os.path.isdir(_p):
        sys.path.insert(0, _p)

import concourse.bacc as bacc
import concourse.tile as tile
import concourse.mybir as mybir
from concourse.bass_utils import run_bass_kernel_spmd

BF16 = ml_dtypes.bfloat16
N_CORES = 8
B_PER = 8           # images per core
MAGIC = float(np.float32(1.5 * 2 ** 23))   # fp32 RNE rounding magic
Alu = mybir.AluOpType
dt = mybir.dt

# tap index k in {0,1,2} -> (parity s, window start offset) for the phase planes
_TAP = {0: (1, 0), 1: (0, 1), 2: (1, 1)}


def _phase_planes(x):
    """(B, C, 28, 28) f32 -> (B, C, 2, 2, 15, 15): plane[sr][sc][q+1][p+1] = x[2q+sr][2p+sc]."""
    B, C = x.shape[:2]
    out = np.zeros((B, C, 2, 2, 15, 15), np.float32)
    for sr in (0, 1):
        for sc in (0, 1):
            out[:, :, sr, sc, 1:15, 1:15] = x[:, :, sr::2, sc::2]
    return out


def _quant_weights(w):
    """Per-tensor int8 narrow-range fake quant; returns (int-valued f32 weights, scale)."""
    s = np.float32(np.max(np.abs(w))) / np.float32(127.0)
    wq = np.clip(np.round(w / s), -127, 127).astype(np.float32)
    return wq, s


def _w_lhsT(w_int, n_ci_blk):
    """(Cout=512, Cin, 3, 3) int-valued -> (ci_blk, 128, 9, 4, 128) bf16 stationary layout."""
    t = w_int.transpose(2, 3, 1, 0)                      # (3, 3, Cin, 512)
    t = t.reshape(9, n_ci_blk, 128, 4, 128)              # (tap, ci_blk, ci_p, co_blk, co)
    return np.ascontiguousarray(t.transpose(1, 2, 0, 3, 4)).astype(BF16)


_skip_ldw = [False]
_orig_InstMatmult = mybir.InstMatmult


def _patched_InstMatmult(*a, **kw):
    if _skip_ldw[0]:
        kw.setdefault("ldweights", False)
    return _orig_InstMatmult(*a, **kw)


def build_program(scale1, scale2, out_scale):
    """Build the (per-core SPMD) Bass program with the given fp32 immediates."""
    nc = bacc.Bacc("TRN2", target_bir_lowering=False, debug=False,
                   num_devices=N_CORES)

    mybir.InstMatmult = _patched_InstMatmult
    try:
        return _build_body(nc, scale1, scale2, out_scale)
    finally:
        mybir.InstMatmult = _orig_InstMatmult


def _build_body(nc, scale1, scale2, out_scale):
    NSP = B_PER * 196
    NT = 4

    xhi_d = nc.dram_tensor("xhi", (128, 2, 4, B_PER, 15, 15), dt.bfloat16, kind="ExternalInput")
    xlo_d = nc.dram_tensor("xlo", (128, 2, 4, B_PER, 15, 15), dt.bfloat16, kind="ExternalInput")
    w1_d = nc.dram_tensor("w1", (2, 128, 9, 4, 128), dt.int8, kind="ExternalInput")
    w2_d = nc.dram_tensor("w2", (4, 128, 9, 4, 128), dt.int8, kind="ExternalInput")
    b1_d = nc.dram_tensor("b1", (128, 4), dt.float32, kind="ExternalInput")
    b2_d = nc.dram_tensor("b2", (128, 4), dt.float32, kind="ExternalInput")
    out_d = nc.dram_tensor("out", (512, B_PER, 196), dt.float32, kind="ExternalOutput")

    def mm(out_ap, w_ap, rhs, start, stop, reuse):
        # reuse=True -> PE keeps the already-loaded stationary weights
        _skip_ldw[0] = reuse
        try:
            nc.tensor.matmul(out_ap, w_ap, rhs, start=start, stop=stop)
        finally:
            _skip_ldw[0] = False

    with tile.TileContext(nc) as tc, ExitStack() as ctx:
        const = ctx.enter_context(tc.tile_pool(name="const", bufs=1))
        psum = ctx.enter_context(tc.tile_pool(name="psum", bufs=8, space="PSUM"))
        tmp = ctx.enter_context(tc.tile_pool(name="tmp", bufs=3))
        outp = ctx.enter_context(tc.tile_pool(name="outp", bufs=3))

        # --- SBUF allocations: one tile per DMA chunk for fine-grained deps ---
        # x planes: [pl][b] -> [128, n, 15, 15]
        xh_t = [const.tile([128, 2, B_PER, 15, 15], dt.bfloat16, tag=f"xh{pl}", name=f"xh{pl}")
                for pl in range(4)]
        xl_t = [const.tile([128, 2, B_PER, 15, 15], dt.bfloat16, tag=f"xl{pl}", name=f"xl{pl}")
                for pl in range(4)]
        # w1: [b] -> [128, tap, co_blk, co]; weights arrive int8, DVE converts
        # them to bf16 (values are integers in [-127,127], exact either way)
        w1_t = [const.tile([128, 9, 4, 128], dt.bfloat16, tag=f"w1{b}", name=f"w1t{b}") for b in range(2)]
        w2_t = [const.tile([128, 9, 4, 128], dt.bfloat16, tag=f"w2{b}", name=f"w2t{b}") for b in range(4)]
        w1i_t = [const.tile([128, 9, 4, 128], dt.int8, tag=f"w1i{b}", name=f"w1i{b}") for b in range(2)]
        w2i_t = [const.tile([128, 9, 4, 128], dt.int8, tag=f"w2i{b}", name=f"w2i{b}") for b in range(4)]
        b1_t = const.tile([128, 4], dt.float32, tag="b1")
        b2_t = const.tile([128, 4], dt.float32, tag="b2")
        act_t = const.tile([128, 4, B_PER, 16, 16], dt.bfloat16, tag="act")  # padded act1
        wz = const.tile([128, 256], dt.bfloat16, tag="wz")

        # PE warm-up source + conv2 padding zeros; act memset on the otherwise
        # idle GpSimd engine so the DVE is free during the input-DMA window.
        nc.vector.memset(wz[:], 0.0)
        nc.gpsimd.memset(act_t[:], 0.0)

        # --- loads in first-use order on the Sync HWDGE queue (outputs use the
        # Scalar queue so they never sit behind input traffic) ---
        def load(dst, src):
            nc.sync.dma_start(out=dst, in_=src)

        def load_plane(pl):
            load(xh_t[pl][:], xhi_d[:, :, pl])
            load(xl_t[pl][:], xlo_d[:, :, pl])

        # w1 first (small), then planes in first-use order 3,2,1,0
        for b in range(2):
            load(w1i_t[b][:], w1_d[b])
            nc.vector.tensor_copy(w1_t[b][:], w1i_t[b][:])
        load_plane(3)
        load(b1_t[:], b1_d[:])
        load_plane(2)
        load_plane(1)
        load_plane(0)
        for b in range(4):
            load(w2i_t[b][:], w2_d[b])
            nc.vector.tensor_copy(w2_t[b][:], w2i_t[b][:])
        load(b2_t[:], b2_d[:])

        def quant_chain(dst, src, sc, bias_ap, width=392):
            """dst = clip(rne(src*sc + bias), -128, 127) on the DVE (3 fused ops)."""
            tt = tmp.tile([128, width], dt.float32, tag=f"tt{min(width, 392)}", name="tt")
            nc.vector.tensor_scalar(tt[:], src, sc, bias_ap, op0=Alu.mult, op1=Alu.add)
            nc.vector.tensor_scalar(tt[:], tt[:], MAGIC, MAGIC + 127.0, op0=Alu.add, op1=Alu.min)
            nc.vector.tensor_scalar(dst, tt[:], MAGIC - 128.0, -MAGIC, op0=Alu.max, op1=Alu.add)
            return tt

        # PE warm-up: junk matmuls on the zeroed tile during the input-DMA wait
        # so the HAM clock gate is at full rate when the real stream starts.
        wps = psum.tile([128, 512], dt.float32, tag="ps", name="warmps")
        for i in range(58):
            nc.tensor.matmul(wps[:, 0:256], wz[:, 0:128], wz[:, 0:256],
                             start=True, stop=True)

        # --- conv1 + act1 ---
        # cb0 is tap-major: plane demand spread over the whole 144-MM group to
        # match DMA delivery. cb1-3 are nt-major: each psum bank finishes early
        # and its epilogue overlaps the remaining banks' matmuls.
        def c1_tap(t9):
            ky, kx = divmod(t9, 3)
            sr, r0 = _TAP[ky]
            sc_, c0 = _TAP[kx]
            return sr * 2 + sc_, r0, c0

        def conv1_group(cb, t9, b, ps_list, nts):
            # one stationary weight (t9, b, cb) serving len(nts) x {hi,lo} matmuls;
            # only the first self-loads the PE array
            pl, r0, c0 = c1_tap(t9)
            w_ap = w1_t[b][:, t9, cb, :]
            first = True
            for src_t in (xh_t, xl_t):
                for i, nt in enumerate(nts):
                    rhs = src_t[pl][:, b, 2 * nt:2 * nt + 2, r0:r0 + 14, c0:c0 + 14]
                    mm(ps_list[i][:, 0:392], w_ap, rhs,
                       start=(t9 == 0 and b == 0 and src_t is xh_t),
                       stop=(t9 == 8 and b == 1 and src_t is xl_t),
                       reuse=not first)
                    first = False

        for cb in range(4):
            if cb == 0:
                # tap-major: plane demand spread over the whole group to match
                # the DMA delivery ramp; 8 matmuls per weight load
                ps_n = [psum.tile([128, 512], dt.float32, tag="ps", name="ps")
                        for _ in range(NT)]
                for t9 in range(9):
                    for b in range(2):
                        conv1_group(cb, t9, b, ps_n, range(NT))
                for nt in range(NT):
                    quant_chain(act_t[:, cb, 2 * nt:2 * nt + 2, 1:15, 1:15],
                                ps_n[nt][:, 0:392], scale1, b1_t[:, cb:cb + 1])
            else:
                # nt-pair-major: each bank pair finishes at half-time so its
                # epilogue overlaps the rest; 4 matmuls per weight load
                for half in range(2):
                    nts = [2 * half, 2 * half + 1]
                    ps_p = [psum.tile([128, 512], dt.float32, tag="ps", name="ps")
                            for _ in nts]
                    for t9 in range(9):
                        for b in range(2):
                            conv1_group(cb, t9, b, ps_p, nts)
                    for i, nt in enumerate(nts):
                        quant_chain(act_t[:, cb, 2 * nt:2 * nt + 2, 1:15, 1:15],
                                    ps_p[i][:, 0:392], scale1, b1_t[:, cb:cb + 1])

        # --- conv2 + act2 (nt-major; per-nt epilogue + out DMA pipeline the tail) ---
        def conv2_epilogue(cb, nt, ps):
            ot = outp.tile([128, 392], dt.float32, tag="ot", name="ot")
            tq_dst = tmp.tile([128, 392], dt.float32, tag="tq", name="tq")
            quant_chain(tq_dst[:], ps[:, 0:392], scale2, b2_t[:, cb:cb + 1])
            nc.vector.tensor_scalar_mul(ot[:], tq_dst[:], out_scale)
            nc.scalar.dma_start(
                out=out_d[cb * 128:(cb + 1) * 128, 2 * nt:2 * nt + 2], in_=ot[:])

        for cb in range(4):
            if cb < 3:
                # tap-major: 4 matmuls per weight load
                ps_n = [psum.tile([128, 512], dt.float32, tag="ps", name="ps")
                        for _ in range(NT)]
                for t9 in range(9):
                    ky, kx = divmod(t9, 3)
                    for b in range(4):
                        w_ap = w2_t[b][:, t9, cb, :]
                        for nt in range(NT):
                            rhs = act_t[:, b, 2 * nt:2 * nt + 2, ky:ky + 14, kx:kx + 14]
                            mm(ps_n[nt][:, 0:392], w_ap, rhs,
                               start=(t9 == 0 and b == 0),
                               stop=(t9 == 8 and b == 3),
                               reuse=nt > 0)
                for nt in range(NT):
                    conv2_epilogue(cb, nt, ps_n[nt])
            else:
                # last block nt-major: each chunk's epilogue + out DMA overlaps
                # the remaining chunks' matmuls; the final two chunks are
                # single-image so the post-stream tail is minimal
                for n0, ncnt in ((0, 2), (2, 2), (4, 2), (6, 1), (7, 1)):
                    w = ncnt * 196
                    ps = psum.tile([128, 512], dt.float32, tag="ps", name="ps")
                    for t9 in range(9):
                        ky, kx = divmod(t9, 3)
                        for b in range(4):
                            w_ap = w2_t[b][:, t9, cb, :]
                            rhs = act_t[:, b, n0:n0 + ncnt, ky:ky + 14, kx:kx + 14]
                            mm(ps[:, 0:w], w_ap, rhs,
                               start=(t9 == 0 and b == 0),
                               stop=(t9 == 8 and b == 3),
                               reuse=False)
                    ot = outp.tile([128, 392], dt.float32, tag="ot", name="ot")
                    tq_dst = tmp.tile([128, 392], dt.float32, tag="tq", name="tq")
                    quant_chain(tq_dst[:, 0:w], ps[:, 0:w], scale2, b2_t[:, cb:cb + 1],
                                width=w)
                    nc.vector.tensor_scalar_mul(ot[:, 0:w], tq_dst[:, 0:w], out_scale)
                    nc.scalar.dma_start(
                        out=out_d[cb * 128:(cb + 1) * 128, n0:n0 + ncnt], in_=ot[:, 0:w])

    _dedupe_ldweights(nc)
    nc.compile()
    return nc


def _dedupe_ldweights(nc):
    """Drop LDWEIGHTS whose stationary operand is identical to the previous
    one on the PE stream (only MATMULs in between): the PE array keeps its
    loaded weights, so consecutive same-weight matmuls need a single load."""
    def sig_of(inst):
        a0 = inst.ins[0]
        try:
            return (a0.memref, a0.offset, str(a0.ap), str(a0.dtype))
        except Exception:
            return None

    removed = 0
    for blk in nc.main_func.blocks:
        last = None
        keep = []
        for inst in blk.instructions:
            tn = type(inst).__name__
            if inst.engine == mybir.EngineType.PE:
                if tn == "InstLdweights":
                    sig = sig_of(inst)
                    si = inst.sync_info
                    clean = si is None or (not si.on_wait and not si.on_update)
                    if sig is not None and sig == last and clean:
                        removed += 1
                        continue
                    last = sig
                elif tn != "InstMatmult":
                    last = None
            keep.append(inst)
        blk.instructions[:] = keep
    return removed


def prepare(x, w1, b1, w2, b2, in_scale, act1_scale, act2_scale):
    """Host-side prep: quantize weights, build per-core input maps + immediates."""
    x = np.asarray(x, np.float32)
    w1 = np.asarray(w1, np.float32)
    b1 = np.asarray(b1, np.float32)
    w2 = np.asarray(w2, np.float32)
    b2 = np.asarray(b2, np.float32)
    s_in = np.float32(np.asarray(in_scale).reshape(-1)[0])
    s_a1 = np.float32(np.asarray(act1_scale).reshape(-1)[0])
    s_a2 = np.float32(np.asarray(act2_scale).reshape(-1)[0])

    w1_int, s_w1 = _quant_weights(w1)
    w2_int, s_w2 = _quant_weights(w2)
    bq1 = np.clip(np.round(b1 / (s_in * s_w1)), -2.0 ** 31, 2.0 ** 31 - 1).astype(np.float32) * (s_in * s_w1)
    bq2 = np.clip(np.round(b2 / (s_a1 * s_w2)), -2.0 ** 31, 2.0 ** 31 - 1).astype(np.float32) * (s_a1 * s_w2)

    scale1 = float(np.float32(s_w1 / s_a1))
    scale2 = float(np.float32(s_a1 * s_w2 / s_a2))
    out_scale = float(s_a2)
    bias1 = np.ascontiguousarray((bq1 / s_a1).astype(np.float32).reshape(4, 128).T)  # (128, 4)
    bias2 = np.ascontiguousarray((bq2 / s_a2).astype(np.float32).reshape(4, 128).T)

    xp = _phase_planes(x)                                  # (64, 256, 2, 2, 15, 15)
    xp_hi = xp.astype(BF16)
    xp_lo = (xp - xp_hi.astype(np.float32)).astype(BF16)

    w1_l = _w_lhsT(w1_int, 2).astype(np.int8)
    w2_l = _w_lhsT(w2_int, 4).astype(np.int8)

    in_maps = []
    for c in range(N_CORES):
        sl = slice(c * B_PER, (c + 1) * B_PER)
        m = {}
        for name, arr in (("xhi", xp_hi[sl]), ("xlo", xp_lo[sl])):
            # (8, 256, 2, 2, 15, 15) -> (ci_blk 2, ci_p 128, plane 4, n 8, 15, 15)
            a = arr.transpose(1, 2, 3, 0, 4, 5).reshape(2, 128, 4, B_PER, 15, 15)
            m[name] = np.ascontiguousarray(a.transpose(1, 0, 2, 3, 4, 5))
        m["w1"] = w1_l
        m["w2"] = w2_l
        m["b1"] = bias1
        m["b2"] = bias2
        in_maps.append(m)
    return (scale1, scale2, out_scale), in_maps


def gather_out(results):
    """Per-core (512, 8, 196) outputs -> full (64, 512, 14, 14)."""
    out = np.empty((N_CORES * B_PER, 512, 14, 14), np.float32)
    for c, r in enumerate(results):
        o = np.asarray(r["out"])                           # (512, 8, 196)
        out[c * B_PER:(c + 1) * B_PER] = o.transpose(1, 0, 2).reshape(B_PER, 512, 14, 14)
    return out


_cache = {}


def kernel(x, w1, b1, w2, b2, in_scale, act1_scale, act2_scale):
    imms, in_maps = prepare(x, w1, b1, w2, b2, in_scale, act1_scale, act2_scale)
    if imms not in _cache:
        _cache[imms] = build_program(*imms)
    nc = _cache[imms]
    res = run_bass_kernel_spmd(nc, in_maps, list(range(N_CORES)))
    return gather_out(res.results)



# revision 8
# speedup vs baseline: 1.2804x; 1.2804x over previous
"""Trainium2 Bass kernel for the quantized BasicBlock (conv3x3/s2 + fakequant + conv3x3/s1 + fakequant).

Sharding: data-parallel over batch across 8 cores (8 images each), weights replicated.

Device math (per core, B=8):
  conv1: implicit GEMM, 9 taps x 2 ci-blocks, input as fp16 (11-bit significand,
         rel err ~4e-3 on final output vs 2e-2 budget), integer-valued fp16 weights
         (exact), fp32 PSUM accum.
  act1:  v = P1*(s_w1/s_a1) + bq1/s_a1; y = clip(rne(v), -128, 127) via the fp32
         magic-number trick on the DVE; y stored as integer-valued bf16 into a
         zero-padded [16x16] layout for conv2.
  conv2: exact integer bf16 GEMM, 9 taps x 4 ci-blocks.
  act2:  v2 = P2*(s_a1*s_w2/s_a2) + bq2/s_a2; out = clip(rne(v2), -128, 127) * s_a2.

Stride-2 conv1 is handled by a host-side phase split: x is scattered into 2x2 parity
planes, zero-padded to 15x15, so each tap reads a stride-1 14x14 window of one plane.

DMAs are split into per-(plane, ci-block) / per-(tap, ci-block) chunks with their own
tiles (dependency granularity) and alternate between the two HWDGE trigger engines
(Sync, Scalar) in first-use order, so the matmul stream starts as soon as tap 0's
weights + plane arrive instead of after the whole input load.
"""
import os
import sys
from contextlib import ExitStack

import numpy as np
import ml_dtypes

for _p in ("/opt/trn_rl_repo",):
    if _p not in sys.path and os.path.isdir(_p):
        sys.path.insert(0, _p)

import concourse.bacc as bacc
import concourse.tile as tile
import concourse.mybir as mybir
from concourse.bass_utils import run_bass_kernel_spmd

BF16 = ml_dtypes.bfloat16
N_CORES = 8
B_PER = 8           # images per core
MAGIC = float(np.float32(1.5 * 2 ** 23))   # fp32 RNE rounding magic
Alu = mybir.AluOpType
dt = mybir.dt

# tap index k in {0,1,2} -> (parity s, window start offset) for the phase planes
_TAP = {0: (1, 0), 1: (0, 1), 2: (1, 1)}


def _phase_planes(x):
    """(B, C, 28, 28) f32 -> (B, C, 2, 2, 15, 15): plane[sr][sc][q+1][p+1] = x[2q+sr][2p+sc]."""
    B, C = x.shape[:2]
    out = np.zeros((B, C, 2, 2, 15, 15), np.float32)
    for sr in (0, 1):
        for sc in (0, 1):
            out[:, :, sr, sc, 1:15, 1:15] = x[:, :, sr::2, sc::2]
    return out


def _quant_weights(w):
    """Per-tensor int8 narrow-range fake quant; returns (int-valued f32 weights, scale)."""
    s = np.float32(np.max(np.abs(w))) / np.float32(127.0)
    wq = np.clip(np.round(w / s), -127, 127).astype(np.float32)
    return wq, s


def _w_lhsT(w_int, n_ci_blk):
    """(Cout=512, Cin, 3, 3) int-valued -> (ci_blk, 128, 9, 4, 128) bf16 stationary layout."""
    t = w_int.transpose(2, 3, 1, 0)                      # (3, 3, Cin, 512)
    t = t.reshape(9, n_ci_blk, 128, 4, 128)              # (tap, ci_blk, ci_p, co_blk, co)
    return np.ascontiguousarray(t.transpose(1, 2, 0, 3, 4)).astype(BF16)


_skip_ldw = [False]
_orig_InstMatmult = mybir.InstMatmult


def _patched_InstMatmult(*a, **kw):
    if _skip_ldw[0]:
        kw.setdefault("ldweights", False)
    return _orig_InstMatmult(*a, **kw)


def build_program(scale1, scale2, out_scale):
    """Build the (per-core SPMD) Bass program with the given fp32 immediates."""
    nc = bacc.Bacc("TRN2", target_bir_lowering=False, debug=False,
                   num_devices=N_CORES)

    mybir.InstMatmult = _patched_InstMatmult
    try:
        return _build_body(nc, scale1, scale2, out_scale)
    finally:
        mybir.InstMatmult = _orig_InstMatmult


def _build_body(nc, scale1, scale2, out_scale):
    NSP = B_PER * 196
    NT = 4

    xhi_d = nc.dram_tensor("xhi", (128, 2, 4, B_PER, 15, 15), dt.float16, kind="ExternalInput")
    w1_d = nc.dram_tensor("w1", (2, 128, 9, 4, 128), dt.int8, kind="ExternalInput")
    w2_d = nc.dram_tensor("w2", (4, 128, 9, 4, 128), dt.int8, kind="ExternalInput")
    b1_d = nc.dram_tensor("b1", (128, 4), dt.float32, kind="ExternalInput")
    b2_d = nc.dram_tensor("b2", (128, 4), dt.float32, kind="ExternalInput")
    out_d = nc.dram_tensor("out", (512, B_PER, 196), dt.float32, kind="ExternalOutput")

    def mm(out_ap, w_ap, rhs, start, stop, reuse):
        # reuse=True -> PE keeps the already-loaded stationary weights
        _skip_ldw[0] = reuse
        try:
            nc.tensor.matmul(out_ap, w_ap, rhs, start=start, stop=stop)
        finally:
            _skip_ldw[0] = False

    with tile.TileContext(nc) as tc, ExitStack() as ctx:
        const = ctx.enter_context(tc.tile_pool(name="const", bufs=1))
        psum = ctx.enter_context(tc.tile_pool(name="psum", bufs=8, space="PSUM"))
        tmp = ctx.enter_context(tc.tile_pool(name="tmp", bufs=3))
        outp = ctx.enter_context(tc.tile_pool(name="outp", bufs=3))

        # --- SBUF allocations: one tile per DMA chunk for fine-grained deps ---
        # x planes: [pl][b] -> [128, n, 15, 15]
        xh_t = [const.tile([128, 2, B_PER, 15, 15], dt.float16, tag=f"xh{pl}", name=f"xh{pl}")
                for pl in range(4)]
        # w1: [b] -> [128, tap, co_blk, co]; weights arrive int8, DVE converts
        # them to fp16/bf16 (values are integers in [-127,127], exact either way)
        w1_t = [const.tile([128, 9, 4, 128], dt.float16, tag=f"w1{b}", name=f"w1t{b}") for b in range(2)]
        w2_t = [const.tile([128, 9, 4, 128], dt.bfloat16, tag=f"w2{b}", name=f"w2t{b}") for b in range(4)]
        w1i_t = [const.tile([128, 9, 4, 128], dt.int8, tag=f"w1i{b}", name=f"w1i{b}") for b in range(2)]
        w2i_t = [const.tile([128, 9, 4, 128], dt.int8, tag=f"w2i{b}", name=f"w2i{b}") for b in range(4)]
        b1_t = const.tile([128, 4], dt.float32, tag="b1")
        b2_t = const.tile([128, 4], dt.float32, tag="b2")
        act_t = const.tile([128, 4, B_PER, 16, 16], dt.bfloat16, tag="act")  # padded act1
        wz = const.tile([128, 256], dt.bfloat16, tag="wz")

        # PE warm-up source + conv2 padding zeros; act memset on the otherwise
        # idle GpSimd engine so the DVE is free during the input-DMA window.
        nc.vector.memset(wz[:], 0.0)
        nc.gpsimd.memset(act_t[:], 0.0)

        # --- loads in first-use order on the Sync HWDGE queue (outputs use the
        # Scalar queue so they never sit behind input traffic) ---
        def load(dst, src):
            nc.sync.dma_start(out=dst, in_=src)

        def load_plane(pl):
            load(xh_t[pl][:], xhi_d[:, :, pl])

        # w1 first (small), then planes in first-use order 3,2,1,0
        for b in range(2):
            load(w1i_t[b][:], w1_d[b])
            nc.vector.tensor_copy(w1_t[b][:], w1i_t[b][:])
        load_plane(3)
        load(b1_t[:], b1_d[:])
        load_plane(2)
        load_plane(1)
        load_plane(0)
        for b in range(4):
            load(w2i_t[b][:], w2_d[b])
            nc.vector.tensor_copy(w2_t[b][:], w2i_t[b][:])
        load(b2_t[:], b2_d[:])

        def quant_chain(dst, src, sc, bias_ap, width=392):
            """dst = clip(rne(src*sc + bias), -128, 127) on the DVE (3 fused ops)."""
            tt = tmp.tile([128, width], dt.float32, tag=f"tt{min(width, 392)}", name="tt")
            nc.vector.tensor_scalar(tt[:], src, sc, bias_ap, op0=Alu.mult, op1=Alu.add)
            nc.vector.tensor_scalar(tt[:], tt[:], MAGIC, MAGIC + 127.0, op0=Alu.add, op1=Alu.min)
            nc.vector.tensor_scalar(dst, tt[:], MAGIC - 128.0, -MAGIC, op0=Alu.max, op1=Alu.add)
            return tt

        # PE warm-up: junk matmuls on the zeroed tile during the input-DMA wait
        # so the HAM clock gate is at full rate when the real stream starts.
        wps = psum.tile([128, 512], dt.float32, tag="ps", name="warmps")
        for i in range(58):
            nc.tensor.matmul(wps[:, 0:256], wz[:, 0:128], wz[:, 0:256],
                             start=True, stop=True)

        # --- conv1 + act1 ---
        # cb0 is tap-major: plane demand spread over the whole 144-MM group to
        # match DMA delivery. cb1-3 are nt-major: each psum bank finishes early
        # and its epilogue overlaps the remaining banks' matmuls.
        def c1_tap(t9):
            ky, kx = divmod(t9, 3)
            sr, r0 = _TAP[ky]
            sc_, c0 = _TAP[kx]
            return sr * 2 + sc_, r0, c0

        def conv1_group(cb, t9, b, ps_list, nts):
            # one stationary weight (t9, b, cb) serving len(nts) matmuls;
            # only the first self-loads the PE array
            pl, r0, c0 = c1_tap(t9)
            w_ap = w1_t[b][:, t9, cb, :]
            for i, nt in enumerate(nts):
                rhs = xh_t[pl][:, b, 2 * nt:2 * nt + 2, r0:r0 + 14, c0:c0 + 14]
                mm(ps_list[i][:, 0:392], w_ap, rhs,
                   start=(t9 == 0 and b == 0),
                   stop=(t9 == 8 and b == 1),
                   reuse=i > 0)

        for cb in range(4):
            if cb == 0:
                # tap-major: plane demand spread over the whole group to match
                # the DMA delivery ramp; 8 matmuls per weight load
                ps_n = [psum.tile([128, 512], dt.float32, tag="ps", name="ps")
                        for _ in range(NT)]
                for t9 in range(9):
                    for b in range(2):
                        conv1_group(cb, t9, b, ps_n, range(NT))
                for nt in range(NT):
                    quant_chain(act_t[:, cb, 2 * nt:2 * nt + 2, 1:15, 1:15],
                                ps_n[nt][:, 0:392], scale1, b1_t[:, cb:cb + 1])
            else:
                # nt-pair-major: each bank pair finishes at half-time so its
                # epilogue overlaps the rest; 4 matmuls per weight load
                for half in range(2):
                    nts = [2 * half, 2 * half + 1]
                    ps_p = [psum.tile([128, 512], dt.float32, tag="ps", name="ps")
                            for _ in nts]
                    for t9 in range(9):
                        for b in range(2):
                            conv1_group(cb, t9, b, ps_p, nts)
                    for i, nt in enumerate(nts):
                        quant_chain(act_t[:, cb, 2 * nt:2 * nt + 2, 1:15, 1:15],
                                    ps_p[i][:, 0:392], scale1, b1_t[:, cb:cb + 1])

        # --- conv2 + act2 (nt-major; per-nt epilogue + out DMA pipeline the tail) ---
        def conv2_epilogue(cb, nt, ps):
            ot = outp.tile([128, 392], dt.float32, tag="ot", name="ot")
            tq_dst = tmp.tile([128, 392], dt.float32, tag="tq", name="tq")
            quant_chain(tq_dst[:], ps[:, 0:392], scale2, b2_t[:, cb:cb + 1])
            nc.vector.tensor_scalar_mul(ot[:], tq_dst[:], out_scale)
            nc.scalar.dma_start(
                out=out_d[cb * 128:(cb + 1) * 128, 2 * nt:2 * nt + 2], in_=ot[:])

        for cb in range(4):
            if cb < 3:
                # tap-major: 4 matmuls per weight load
                ps_n = [psum.tile([128, 512], dt.float32, tag="ps", name="ps")
                        for _ in range(NT)]
                for t9 in range(9):
                    ky, kx = divmod(t9, 3)
                    for b in range(4):
                        w_ap = w2_t[b][:, t9, cb, :]
                        for nt in range(NT):
                            rhs = act_t[:, b, 2 * nt:2 * nt + 2, ky:ky + 14, kx:kx + 14]
                            mm(ps_n[nt][:, 0:392], w_ap, rhs,
                               start=(t9 == 0 and b == 0),
                               stop=(t9 == 8 and b == 3),
                               reuse=nt > 0)
                for nt in range(NT):
                    conv2_epilogue(cb, nt, ps_n[nt])
            else:
                # last block nt-major: each chunk's epilogue + out DMA overlaps
                # the remaining chunks' matmuls; the final two chunks are
                # single-image so the post-stream tail is minimal
                for n0, ncnt in ((0, 2), (2, 2), (4, 2), (6, 1), (7, 1)):
                    w = ncnt * 196
                    ps = psum.tile([128, 512], dt.float32, tag="ps", name="ps")
                    for t9 in range(9):
                        ky, kx = divmod(t9, 3)
                        for b in range(4):
                            w_ap = w2_t[b][:, t9, cb, :]
                            rhs = act_t[:, b, n0:n0 + ncnt, ky:ky + 14, kx:kx + 14]
                            mm(ps[:, 0:w], w_ap, rhs,
                               start=(t9 == 0 and b == 0),
                               stop=(t9 == 8 and b == 3),
                               reuse=False)
                    ot = outp.tile([128, 392], dt.float32, tag="ot", name="ot")
                    tq_dst = tmp.tile([128, 392], dt.float32, tag="tq", name="tq")
                    quant_chain(tq_dst[:, 0:w], ps[:, 0:w], scale2, b2_t[:, cb:cb + 1],
                                width=w)
                    nc.vector.tensor_scalar_mul(ot[:, 0:w], tq_dst[:, 0:w], out_scale)
                    nc.scalar.dma_start(
                        out=out_d[cb * 128:(cb + 1) * 128, n0:n0 + ncnt], in_=ot[:, 0:w])

    _dedupe_ldweights(nc)
    nc.compile()
    return nc


def _dedupe_ldweights(nc):
    """Drop LDWEIGHTS whose stationary operand is identical to the previous
    one on the PE stream (only MATMULs in between): the PE array keeps its
    loaded weights, so consecutive same-weight matmuls need a single load."""
    def sig_of(inst):
        a0 = inst.ins[0]
        try:
            return (a0.memref, a0.offset, str(a0.ap), str(a0.dtype))
        except Exception:
            return None

    removed = 0
    for blk in nc.main_func.blocks:
        last = None
        keep = []
        for inst in blk.instructions:
            tn = type(inst).__name__
            if inst.engine == mybir.EngineType.PE:
                if tn == "InstLdweights":
                    sig = sig_of(inst)
                    si = inst.sync_info
                    clean = si is None or (not si.on_wait and not si.on_update)
                    if sig is not None and sig == last and clean:
                        removed += 1
                        continue
                    last = sig
                elif tn != "InstMatmult":
                    last = None
            keep.append(inst)
        blk.instructions[:] = keep
    return removed


def prepare(x, w1, b1, w2, b2, in_scale, act1_scale, act2_scale):
    """Host-side prep: quantize weights, build per-core input maps + immediates."""
    x = np.asarray(x, np.float32)
    w1 = np.asarray(w1, np.float32)
    b1 = np.asarray(b1, np.float32)
    w2 = np.asarray(w2, np.float32)
    b2 = np.asarray(b2, np.float32)
    s_in = np.float32(np.asarray(in_scale).reshape(-1)[0])
    s_a1 = np.float32(np.asarray(act1_scale).reshape(-1)[0])
    s_a2 = np.float32(np.asarray(act2_scale).reshape(-1)[0])

    w1_int, s_w1 = _quant_weights(w1)
    w2_int, s_w2 = _quant_weights(w2)
    bq1 = np.clip(np.round(b1 / (s_in * s_w1)), -2.0 ** 31, 2.0 ** 31 - 1).astype(np.float32) * (s_in * s_w1)
    bq2 = np.clip(np.round(b2 / (s_a1 * s_w2)), -2.0 ** 31, 2.0 ** 31 - 1).astype(np.float32) * (s_a1 * s_w2)

    scale1 = float(np.float32(s_w1 / s_a1))
    scale2 = float(np.float32(s_a1 * s_w2 / s_a2))
    out_scale = float(s_a2)
    bias1 = np.ascontiguousarray((bq1 / s_a1).astype(np.float32).reshape(4, 128).T)  # (128, 4)
    bias2 = np.ascontiguousarray((bq2 / s_a2).astype(np.float32).reshape(4, 128).T)

    xp = _phase_planes(x)                                  # (64, 256, 2, 2, 15, 15)
    xp_hi = xp.astype(np.float16)

    w1_l = _w_lhsT(w1_int, 2).astype(np.int8)
    w2_l = _w_lhsT(w2_int, 4).astype(np.int8)

    in_maps = []
    for c in range(N_CORES):
        sl = slice(c * B_PER, (c + 1) * B_PER)
        m = {}
        for name, arr in (("xhi", xp_hi[sl]),):
            # (8, 256, 2, 2, 15, 15) -> (ci_blk 2, ci_p 128, plane 4, n 8, 15, 15)
            a = arr.transpose(1, 2, 3, 0, 4, 5).reshape(2, 128, 4, B_PER, 15, 15)
            m[name] = np.ascontiguousarray(a.transpose(1, 0, 2, 3, 4, 5))
        m["w1"] = w1_l
        m["w2"] = w2_l
        m["b1"] = bias1
        m["b2"] = bias2
        in_maps.append(m)
    return (scale1, scale2, out_scale), in_maps


def gather_out(results):
    """Per-core (512, 8, 196) outputs -> full (64, 512, 14, 14)."""
    out = np.empty((N_CORES * B_PER, 512, 14, 14), np.float32)
    for c, r in enumerate(results):
        o = np.asarray(r["out"])                           # (512, 8, 196)
        out[c * B_PER:(c + 1) * B_PER] = o.transpose(1, 0, 2).reshape(B_PER, 512, 14, 14)
    return out


_cache = {}


def kernel(x, w1, b1, w2, b2, in_scale, act1_scale, act2_scale):
    imms, in_maps = prepare(x, w1, b1, w2, b2, in_scale, act1_scale, act2_scale)
    if imms not in _cache:
        _cache[imms] = build_program(*imms)
    nc = _cache[imms]
    res = run_bass_kernel_spmd(nc, in_maps, list(range(N_CORES)))
    return gather_out(res.results)

